# revision 6
# baseline (speedup 1.0000x reference)
"""CoupledClustersLossV2 Trainium2 kernel (v2 — wide-DMA layout).

Full inputs in, full output out. Shards embeddings [16384, 2048] f32
across 8 NeuronCores along the class axis (each core: 32 classes = 2048
rows), computes per-class losses on-core, means on the host.

v2 layout: each SBUF partition holds R=4 consecutive HBM rows, so every
DMA packet is a 32 KiB contiguous HBM read (vs 8 KiB row-packets in v1)
— the 16 DMA engines were packet-rate-bound at ~17 GB/s each.

Per-core pipeline (4 supertiles of [128 partitions, R=4 rows, D=2048]):
  - anchor[m, d] = mean of the 32 pos rows of m's class, computed as 4
    PSUM-accumulated matmuls with a constant A[p, m] = 1/32 * [p in pos
    partitions of m's class] (class = 16 consecutive partitions).
  - diff_r = x_r - anchor on VectorE; d2[:, t] = sum_d diff_r^2 via
    ScalarE Square+accum (last slot on VectorE mult+reduce for balance).
  - Tail: SBUF->SBUF DMA gather permutes d2 [128, 16] into class-major
    [32 classes, 64 rows]; then sqrt, per-class min / hinge / sum are
    all free-dim ops. Losses [32] DMA'd out per core.
"""

import sys

import numpy as np

for _p in ("/opt/trn_rl_repo",):
    if _p not in sys.path:
        sys.path.append(_p)

import concourse.bacc as bacc
import concourse.mybir as mybir
from concourse import tile
from concourse.bass_utils import run_bass_kernel_spmd

N_CORES = 8
D = 2048
S = 32                  # samples per class per polarity
ROWS_PER_CLASS = 2 * S  # 64: 32 pos then 32 neg
C_PER_CORE = 32         # classes per core (256 / 8)
ROWS_PER_CORE = C_PER_CORE * ROWS_PER_CLASS  # 2048

R = 4                   # HBM rows per SBUF partition (DMA packet = R*8KiB)
NS = ROWS_PER_CORE // (128 * R)              # supertiles per core
PPC = ROWS_PER_CLASS // R                    # partitions per class
CPS = 128 // PPC                             # classes per supertile
NCOL = ROWS_PER_CORE // 128                  # d2 columns (16)
MM_CHUNK = 512                               # matmul free-dim (1 PSUM bank)
V_SQ = 1                # slots per supertile squared on VectorE (rest ScalarE)
DMA_MODE = "alt"        # input loads: alternate sync/gpsimd queues
TRACE = False

F32 = mybir.dt.float32
F32R = mybir.dt.float32r
AF = mybir.ActivationFunctionType
ALU = mybir.AluOpType

_CACHE = {}
LAST_RESULTS = None


def _a_matrix() -> np.ndarray:
    """A[p, m] = 1/32 if p is a pos partition of m's class.

    Class of partition q = q // PPC; pos partitions are the first PPC/2
    of each class (row-within-class = R*(q % PPC) + r < 32 for all r).
    """
    a = np.zeros((128, 128), dtype=np.float32)
    for m in range(128):
        c = m // PPC
        for pj in range(PPC // 2):
            a[c * PPC + pj, m] = 1.0 / S
    return a


def _inline_tensor(nc, data: np.ndarray, name: str, dtype):
    """nc.inline_tensor with an explicit BIR dtype (e.g. float32r over
    float32 bytes — same width, so the embedded .npy payload is valid)."""
    import base64
    import io

    import concourse.bass as bass

    data = np.ascontiguousarray(data)
    assert mybir.dt.size(dtype) == data.dtype.itemsize
    mls = nc._tensor(name, list(data.shape), dtype, kind="Const", type="DRAM")
    buf = io.BytesIO()
    np.save(buf, data, allow_pickle=False)
    mls.file = f"{name}.npy"
    mls.ant_data = base64.standard_b64encode(buf.getvalue()).decode()
    return bass.DRamTensorHandle(name, list(data.shape), dtype)


def _build(margin: float):
    nc = bacc.Bacc("TRN2", target_bir_lowering=False, debug=False)
    emb = nc.dram_tensor("emb", [ROWS_PER_CORE, D], F32R, kind="ExternalInput")
    out = nc.dram_tensor("losses", [C_PER_CORE], F32, kind="ExternalOutput")
    scratch = nc.dram_tensor("d2scratch", [ROWS_PER_CORE], F32, kind="Internal")

    a_const = _inline_tensor(nc, _a_matrix(), "amat", F32R)

    with tile.TileContext(nc) as tc:
        with (
            tc.tile_pool(name="consts", bufs=1) as cpool,
            tc.tile_pool(name="stats", bufs=1) as spool,
            tc.tile_pool(name="inp", bufs=3) as ipool,
            tc.tile_pool(name="dif", bufs=3) as dpool,
        ):
            a_sb = cpool.tile([128, 128], F32R)
            nc.gpsimd.dma_start(out=a_sb[:], in_=a_const[:, :])
            d2 = spool.tile([128, NCOL], F32)

            # supertile s = rows [128*R*s, 128*R*(s+1)): [128, (r d)] with
            # partition p <- rows R*p .. R*p+R-1 (32 KiB contiguous each)
            emb_sv = emb[:, :].rearrange(
                "(s p r) d -> s p (r d)", s=NS, p=128, r=R
            )

            with tc.tile_pool(name="panc", bufs=2, space="PSUM") as ppool:
                for s_ in range(NS):
                    x = ipool.tile([128, R * D], F32R)
                    eng = nc.sync if s_ % 2 == 0 else nc.gpsimd
                    if DMA_MODE == "hwdge":
                        eng = nc.sync
                    elif DMA_MODE == "swdge":
                        eng = nc.gpsimd
                    eng.dma_start(out=x[:], in_=emb_sv[s_])

                    anchor = ppool.tile([128, D], F32)
                    for c in range(D // MM_CHUNK):
                        for r in range(R):
                            nc.tensor.matmul(
                                anchor[:, MM_CHUNK * c : MM_CHUNK * (c + 1)],
                                a_sb[:],
                                x[:, r * D + MM_CHUNK * c : r * D + MM_CHUNK * (c + 1)],
                                start=(r == 0),
                                stop=(r == R - 1),
                            )
                    for r in range(R):
                        t = R * s_ + r
                        xr = x[:, r * D : (r + 1) * D].bitcast(F32)
                        diff = dpool.tile([128, D], F32, tag=f"diff{r % 2}")
                        nc.vector.tensor_tensor(
                            diff[:], xr, anchor[:], op=ALU.subtract
                        )
                        if r >= R - V_SQ:
                            sq = dpool.tile([128, D], F32, tag="vsq")
                            nc.vector.tensor_tensor(
                                sq[:], diff[:], diff[:], op=ALU.mult
                            )
                            nc.vector.tensor_reduce(
                                d2[:, t : t + 1],
                                sq[:],
                                axis=mybir.AxisListType.X,
                                op=ALU.add,
                            )
                        else:
                            nc.scalar.activation(
                                diff[:],
                                diff[:],
                                AF.Square,
                                accum_out=d2[:, t : t + 1],
                            )

            with tc.tile_pool(name="tail", bufs=1) as tpool:
                # ACT table preload: get Sqrt loaded while the gather DMA
                # runs. Input depends on a late d2 column so the scheduler
                # keeps it near the end of the Square stream.
                warm = tpool.tile([1, 1], F32)
                nc.scalar.activation(
                    warm[:], d2[0:1, R * (NS - 1) : R * (NS - 1) + 1], AF.Sqrt
                )

                # class-major gather via DRAM round-trip (SBUF APs cannot
                # move partition components into free dims): d2 [p, (s r)]
                # -> scratch[row] with row = 128*R*s + R*p + r, then load
                # back as [class, row-within-class].
                d2c = tpool.tile([C_PER_CORE, ROWS_PER_CLASS], F32)
                gdst = scratch[:].rearrange("(s p r) -> p s r", s=NS, p=128, r=R)
                nc.sync.dma_start(
                    out=gdst,
                    in_=d2[:, :].rearrange("p (s r) -> p s r", s=NS, r=R),
                )
                nc.sync.dma_start(
                    out=d2c[:],
                    in_=scratch[:].rearrange("(c i) -> c i", c=C_PER_CORE),
                )

                dist = tpool.tile([C_PER_CORE, ROWS_PER_CLASS], F32)
                nc.scalar.activation(dist[:], d2c[:], AF.Sqrt)
                an = tpool.tile([C_PER_CORE, 1], F32)
                nc.vector.tensor_reduce(
                    an[:], dist[:, S:], axis=mybir.AxisListType.X, op=ALU.min
                )
                anm = tpool.tile([C_PER_CORE, 1], F32)
                nc.vector.tensor_scalar(
                    anm[:], an[:], float(margin), None, op0=ALU.subtract
                )
                hinge = tpool.tile([C_PER_CORE, S], F32)
                nc.vector.tensor_scalar(
                    hinge[:], dist[:, :S], anm[:], 0.0, op0=ALU.subtract, op1=ALU.max
                )
                hsq = tpool.tile([C_PER_CORE, S], F32)
                nc.vector.tensor_tensor(hsq[:], hinge[:], hinge[:], op=ALU.mult)
                losses = tpool.tile([C_PER_CORE, 1], F32)
                nc.vector.tensor_reduce(
                    losses[:], hsq[:], axis=mybir.AxisListType.X, op=ALU.add
                )
                nc.sync.dma_start(out=out[:], in_=losses[:, 0])

    nc.compile()
    return nc


def kernel(embeddings, target=None, margin=0.3, n_classes=256, n_samples=32, **_):
    global LAST_RESULTS
    emb = np.ascontiguousarray(np.asarray(embeddings, dtype=np.float32))
    assert emb.shape == (16384, 2048), emb.shape
    assert int(n_classes) == 256 and int(n_samples) == 32

    key = (float(margin), R, V_SQ, DMA_MODE)
    nc = _CACHE.get(key)
    if nc is None:
        nc = _CACHE[key] = _build(float(margin))

    shards = emb.reshape(N_CORES, ROWS_PER_CORE, D)
    in_maps = [{"emb": shards[c]} for c in range(N_CORES)]
    res = run_bass_kernel_spmd(
        nc, in_maps, core_ids=list(range(N_CORES)), trace=TRACE
    )
    LAST_RESULTS = res
    per_class = np.concatenate([r["losses"].reshape(-1) for r in res.results])
    return np.float32(per_class.mean())


# revision 9
# speedup vs baseline: 1.1039x; 1.1039x over previous
"""CoupledClustersLossV2 Trainium2 kernel (v3 — wide-DMA, variable supertiles).

Full inputs in, full output out. Shards embeddings [16384, 2048] f32
across 8 NeuronCores along the class axis (each core: 32 classes = 2048
rows), computes per-class losses on-core, means on the host.

Layout: each SBUF partition holds R consecutive HBM rows, so DMA packets
are R*8KiB contiguous HBM reads (the 16 DMA engines are packet-rate
bound: 8K->17, 16K->~23, 32K->26 GB/s each). R varies per supertile
(R_SEQ): small first supertile starts compute early, small last ones
shrink the pipeline drain.

Per-core pipeline, per supertile [128 partitions, R rows, D=2048]:
  - anchor[m, d] = mean of the 32 pos rows of m's class via R
    PSUM-accumulated matmuls with constant A_R[p, m] = 1/32 * [p pos
    partition of m's class]  (class = 64/R consecutive partitions).
  - diff_r = x_r - anchor: VectorE (last slot of R>=2 tiles: GpSimd).
  - d2[:, t] = sum_d diff_r^2: ScalarE Square+accum (all slots).
  - d2 cols stream to a DRAM scratch in row-order per supertile.
Tail: read scratch back class-major [32, 64]; sqrt, per-class min /
hinge / sum are free-dim ops; losses [32] DMA'd out.
"""

import sys

import numpy as np

for _p in ("/opt/trn_rl_repo",):
    if _p not in sys.path:
        sys.path.append(_p)

import concourse.bacc as bacc
import concourse.mybir as mybir
from concourse import tile
from concourse.bass_utils import run_bass_kernel_spmd

N_CORES = 8
D = 2048
S = 32                  # samples per class per polarity
ROWS_PER_CLASS = 2 * S  # 64: 32 pos then 32 neg
C_PER_CORE = 32         # classes per core (256 / 8)
ROWS_PER_CORE = C_PER_CORE * ROWS_PER_CLASS  # 2048

R_SEQ = (2, 4, 4, 4, 1, 1)  # rows-per-partition per supertile; sum*128 = 2048
MM_CHUNK = 512              # matmul free-dim (1 PSUM bank)
DMA_MODE = "hwdge"          # input loads: sync HWDGE only (gpsimd does subs)
TRACE = False

F32 = mybir.dt.float32
F32R = mybir.dt.float32r
AF = mybir.ActivationFunctionType
ALU = mybir.AluOpType

_CACHE = {}
LAST_RESULTS = None


def _a_matrix(r: int) -> np.ndarray:
    """A_r[p, m] = 1/32 if p is a pos partition of m's class (R=r layout).

    Class of partition q = q // (64/r); pos partitions are the first
    32/r of each class block.
    """
    ppc = ROWS_PER_CLASS // r
    a = np.zeros((128, 128), dtype=np.float32)
    for m in range(128):
        c = m // ppc
        for pj in range(ppc // 2):
            a[c * ppc + pj, m] = 1.0 / S
    return a


def _inline_tensor(nc, data: np.ndarray, name: str, dtype):
    """nc.inline_tensor with an explicit BIR dtype (e.g. float32r over
    float32 bytes — same width, so the embedded .npy payload is valid)."""
    import base64
    import io

    import concourse.bass as bass

    data = np.ascontiguousarray(data)
    assert mybir.dt.size(dtype) == data.dtype.itemsize
    mls = nc._tensor(name, list(data.shape), dtype, kind="Const", type="DRAM")
    buf = io.BytesIO()
    np.save(buf, data, allow_pickle=False)
    mls.file = f"{name}.npy"
    mls.ant_data = base64.standard_b64encode(buf.getvalue()).decode()
    return bass.DRamTensorHandle(name, list(data.shape), dtype)


def _build(margin: float):
    assert sum(R_SEQ) * 128 == ROWS_PER_CORE
    nc = bacc.Bacc("TRN2", target_bir_lowering=False, debug=False)
    emb = nc.dram_tensor("emb", [ROWS_PER_CORE, D], F32R, kind="ExternalInput")
    out = nc.dram_tensor("losses", [C_PER_CORE], F32, kind="ExternalOutput")
    scratch = nc.dram_tensor("d2scratch", [ROWS_PER_CORE], F32, kind="Internal")

    a_consts = {
        r: _inline_tensor(nc, _a_matrix(r), f"amat{r}", F32R)
        for r in sorted(set(R_SEQ))
    }

    with tile.TileContext(nc) as tc:
        with (
            tc.tile_pool(name="consts", bufs=1) as cpool,
            tc.tile_pool(name="stats", bufs=1) as spool,
            tc.tile_pool(name="inp", bufs=4) as ipool,
            tc.tile_pool(name="dif", bufs=4) as dpool,
        ):
            a_sb = {}
            for r, hnd in a_consts.items():
                a_sb[r] = cpool.tile(
                    [128, 128], F32R, tag=f"amat{r}", name=f"amat{r}_sb"
                )
                nc.gpsimd.dma_start(out=a_sb[r][:], in_=hnd[:, :])
            ncol = sum(R_SEQ)
            d2 = spool.tile([128, ncol], F32)

            emb_flat = emb[:, :].rearrange("n d -> (n d)")

            with tc.tile_pool(name="panc", bufs=2, space="PSUM") as ppool:
                base = 0  # row offset of current supertile
                col = 0   # d2 column offset
                for s_, R in enumerate(R_SEQ):
                    # partition p <- rows base + R*p .. base + R*p + R-1
                    src = emb_flat[D * base : D * (base + 128 * R)].rearrange(
                        "(p q) -> p q", p=128
                    )
                    x = ipool.tile([128, R * D], F32R, tag="x")
                    eng = nc.gpsimd if DMA_MODE == "swdge" else nc.sync
                    if DMA_MODE == "alt":
                        eng = nc.sync if s_ % 2 == 0 else nc.gpsimd
                    eng.dma_start(out=x[:, : R * D], in_=src)

                    anchor = ppool.tile([128, D], F32)
                    for c in range(D // MM_CHUNK):
                        lo = MM_CHUNK * c
                        hi = MM_CHUNK * (c + 1)
                        for r in range(R):
                            nc.tensor.matmul(
                                anchor[:, lo:hi],
                                a_sb[R][:],
                                x[:, r * D + lo : r * D + hi],
                                start=(r == 0),
                                stop=(r == R - 1),
                            )
                    for r in range(R):
                        xr = x[:, r * D : (r + 1) * D].bitcast(F32)
                        diff = dpool.tile([128, D], F32, tag="diff")
                        # (GpSimd cannot read PSUM, so all subs go on DVE)
                        nc.vector.tensor_tensor(diff[:], xr, anchor[:], op=ALU.subtract)
                        nc.scalar.activation(
                            diff[:],
                            diff[:],
                            AF.Square,
                            accum_out=d2[:, col + r : col + r + 1],
                        )
                    # stream this supertile's d2 columns to DRAM in row order
                    nc.sync.dma_start(
                        out=scratch[base : base + 128 * R].rearrange(
                            "(p r) -> p r", p=128
                        ),
                        in_=d2[:, col : col + R],
                    )
                    base += 128 * R
                    col += R

            with tc.tile_pool(name="tail", bufs=1) as tpool:
                # ACT table preload: pull Sqrt in while the last squares
                # retire; input depends on a late d2 column so the
                # scheduler keeps it near the end of the Square stream.
                warm = tpool.tile([1, 1], F32)
                nc.scalar.activation(
                    warm[:], d2[0:1, ncol - 1 : ncol], AF.Sqrt
                )

                d2c = tpool.tile([C_PER_CORE, ROWS_PER_CLASS], F32)
                nc.sync.dma_start(
                    out=d2c[:],
                    in_=scratch[:].rearrange("(c i) -> c i", c=C_PER_CORE),
                )
                dist = tpool.tile([C_PER_CORE, ROWS_PER_CLASS], F32)
                nc.scalar.activation(dist[:], d2c[:], AF.Sqrt)
                an = tpool.tile([C_PER_CORE, 1], F32)
                nc.vector.tensor_reduce(
                    an[:], dist[:, S:], axis=mybir.AxisListType.X, op=ALU.min
                )
                anm = tpool.tile([C_PER_CORE, 1], F32)
                nc.vector.tensor_scalar(
                    anm[:], an[:], float(margin), None, op0=ALU.subtract
                )
                hinge = tpool.tile([C_PER_CORE, S], F32)
                nc.vector.tensor_scalar(
                    hinge[:], dist[:, :S], anm[:], 0.0, op0=ALU.subtract, op1=ALU.max
                )
                hsq = tpool.tile([C_PER_CORE, S], F32)
                nc.vector.tensor_tensor(hsq[:], hinge[:], hinge[:], op=ALU.mult)
                losses = tpool.tile([C_PER_CORE, 1], F32)
                nc.vector.tensor_reduce(
                    losses[:], hsq[:], axis=mybir.AxisListType.X, op=ALU.add
                )
                nc.sync.dma_start(out=out[:], in_=losses[:, 0])

    nc.compile()
    return nc


def kernel(embeddings, target=None, margin=0.3, n_classes=256, n_samples=32, **_):
    global LAST_RESULTS
    emb = np.ascontiguousarray(np.asarray(embeddings, dtype=np.float32))
    assert emb.shape == (16384, 2048), emb.shape
    assert int(n_classes) == 256 and int(n_samples) == 32

    key = (float(margin), R_SEQ, DMA_MODE)
    nc = _CACHE.get(key)
    if nc is None:
        nc = _CACHE[key] = _build(float(margin))

    shards = emb.reshape(N_CORES, ROWS_PER_CORE, D)
    in_maps = [{"emb": shards[c]} for c in range(N_CORES)]
    res = run_bass_kernel_spmd(
        nc, in_maps, core_ids=list(range(N_CORES)), trace=TRACE
    )
    LAST_RESULTS = res
    per_class = np.concatenate([r["losses"].reshape(-1) for r in res.results])
    return np.float32(per_class.mean())
